# revision 1
# baseline (speedup 1.0000x reference)
"""Trainium2 Bass kernel: MoE top-k router (top-8 of 64 experts + softmax).

Contract: kernel(logits, top_k) takes the FULL inputs (logits [1048576, 64]
f32, top_k == 8) and returns (topk_idx int64 [N, 8], topk_w f32 [N, 8]),
matching jax.lax.top_k + jax.nn.softmax semantics (stable descending order,
ties broken toward the smaller index).

Sharding: data-parallel over tokens across 8 NeuronCores (one SPMD program,
per-core slices fed via run_bass_kernel_spmd). Per core, tokens are laid out
partition-major — partition p owns tokens [p*1024, (p+1)*1024) — so every
DMA moves contiguous multi-KB runs per partition.

Per 128-token group the DVE executes one MAX8 (top-8 values, descending,
exact f32 compare) and one MATCH_VALUE_LOAD+FIND_INDEX8 (stable first-match
indices; the HW match unit skips already-matched positions, so duplicate
values get distinct indices in jax order). These three DVE ops are the
bottleneck (~320 ns per 128 tokens); everything else is kept off the DVE:
exp on ScalarE, softmax-denominator tree-sum and the final scale multiply
on GPSIMD. The reciprocal runs on DVE via the 2-instruction ~2ULP
Newton-Raphson approximation (cheaper than the iterative-divide op).
MAX8s are issued in a phase before the FIND pairs so the DVE streams
back-to-back at its ~126 ns/instruction floor. A small first tile (16
tokens/partition) lets the DVE start before the first full 2 MiB tile lands.
"""

import sys

if "/opt/trn_rl_repo" not in sys.path:
    sys.path.insert(0, "/opt/trn_rl_repo")

import numpy as np

N_TOKENS = 1048576
E = 64             # experts
K = 8              # top-k
NCORES = 8
P = 128            # SBUF partitions
TPC = N_TOKENS // NCORES   # tokens per core = 131072
TPP = TPC // P             # tokens per partition = 1024
T = 64                     # tokens per partition per full tile
RAMP = 16                  # first-tile size (earlier DVE start)

_CACHE = {}


def _build(tpp=TPP, t_tile=T, ramp=RAMP):
    import concourse.bacc as bacc
    import concourse.mybir as mybir
    import concourse.tile as tile

    f32 = mybir.dt.float32
    u16 = mybir.dt.uint16

    n_tok = P * tpp
    # graduated small tiles at the start (DVE begins after a 128KB load
    # instead of 2MB) and a small last tile (shorter softmax/store tail
    # after the final DVE instruction)
    if ramp and tpp > 3 * t_tile:
        sizes = ([4, 12, 48] + [t_tile] * (tpp // t_tile - 2)
                 + [t_tile - 16, 16])
    else:
        sizes = [t_tile] * (tpp // t_tile)
    assert sum(sizes) == tpp
    offs = [sum(sizes[:j]) for j in range(len(sizes))]

    nc = bacc.Bacc("TRN2", target_bir_lowering=False, debug=False)
    logits = nc.dram_tensor("logits", [n_tok, E], f32, kind="ExternalInput")
    idx_out = nc.dram_tensor("idx_out", [n_tok, K], u16, kind="ExternalOutput")
    w_out = nc.dram_tensor("w_out", [n_tok, K], f32, kind="ExternalOutput")

    # partition-major: token(p, t) = p*tpp + t
    lg_v = logits.ap().rearrange("(p t) e -> p t e", p=P, t=tpp)
    ix_v = idx_out.ap().rearrange("(p t) k -> p t k", p=P, t=tpp)
    w_v = w_out.ap().rearrange("(p t) k -> p t k", p=P, t=tpp)

    with tile.TileContext(nc) as tc:
        with tc.tile_pool(name="io", bufs=4) as pool:
            for o, tt in zip(offs, sizes):
                x = pool.tile([P, tt, E], f32, tag="x")
                nc.sync.dma_start(x[:], lg_v[:, o:o + tt, :])
                vals = pool.tile([P, tt, K], f32, tag="vals")
                idx = pool.tile([P, tt, K], u16, tag="idx")
                # phase order: all MAX8 first, then the FIND pairs — the DVE
                # then streams each opcode back-to-back without RAW stalls
                for t in range(tt):
                    nc.vector.max(vals[:, t, :], x[:, t, :])
                for t in range(tt):
                    nc.vector.max_index(idx[:, t, :], vals[:, t, :], x[:, t, :])
                ex = pool.tile([P, tt, K], f32, tag="ex")
                nc.scalar.activation(
                    ex[:], vals[:], mybir.ActivationFunctionType.Exp
                )
                # softmax denominator: pairwise tree-sum on GPSIMD (keeps
                # the DVE free for MAX8/FIND_INDEX8, its bottleneck)
                t1 = pool.tile([P, tt, 4], f32, tag="t1")
                t2 = pool.tile([P, tt, 2], f32, tag="t2")
                s = pool.tile([P, tt, 1], f32, tag="s")
                nc.gpsimd.tensor_add(t1[:], ex[:, :, 0:4], ex[:, :, 4:8])
                nc.gpsimd.tensor_add(t2[:], t1[:, :, 0:2], t1[:, :, 2:4])
                nc.gpsimd.tensor_add(s[:], t2[:, :, 0:1], t2[:, :, 1:2])
                # reciprocal: 1-instruction ~51ULP seed on DVE, then one
                # Newton-Raphson refinement (~2ULP) on GPSIMD — only the
                # seed touches the bottleneck engine
                r = pool.tile([P, tt, 1], f32, tag="r")
                if o + tt == tpp:
                    # last tile: its reciprocal chain is serial tail after the
                    # final DVE op — one exact DVE reciprocal (FD=16) beats
                    # seed + 3 GPSIMD refinement ops there
                    nc.vector.reciprocal(r[:], s[:])
                else:
                    rs = pool.tile([P, tt, 1], f32, tag="rs")
                    nc.vector.reciprocal_approx_fast(rs[:], s[:])
                    pq = pool.tile([P, tt, 1], f32, tag="pq")
                    nc.gpsimd.tensor_mul(pq[:], s[:], rs[:])
                    nc.gpsimd.tensor_scalar(pq[:], pq[:], -1.0, 2.0,
                                            op0=mybir.AluOpType.mult,
                                            op1=mybir.AluOpType.add)
                    nc.gpsimd.tensor_mul(r[:], rs[:], pq[:])
                w = pool.tile([P, tt, K], f32, tag="w")
                nc.gpsimd.tensor_mul(w[:], ex[:], r[:].broadcast_to([P, tt, K]))
                nc.sync.dma_start(ix_v[:, o:o + tt, :], idx[:])
                nc.sync.dma_start(w_v[:, o:o + tt, :], w[:])
    nc.compile()
    return nc


def _get_nc():
    if "nc" not in _CACHE:
        _CACHE["nc"] = _build()
    return _CACHE["nc"]


def kernel(logits, top_k):
    logits = np.asarray(logits, dtype=np.float32)
    k = int(np.asarray(top_k))
    assert k == K, f"kernel hardcodes top_k={K}, got {k}"
    assert logits.shape == (N_TOKENS, E), logits.shape

    from concourse.bass_utils import run_bass_kernel_spmd

    nc = _get_nc()
    chunks = logits.reshape(NCORES, TPC, E)
    in_maps = [{"logits": np.ascontiguousarray(chunks[c])} for c in range(NCORES)]
    # The tunneled devices occasionally fail a run with a transient
    # NRT_EXEC_UNIT_UNRECOVERABLE error; a straight retry recovers.
    last_err = None
    for _attempt in range(3):
        try:
            res = run_bass_kernel_spmd(nc, in_maps, list(range(NCORES)))
            break
        except Exception as e:  # noqa: BLE001 - retry transient device faults
            last_err = e
            import time as _time

            _time.sleep(5.0)
    else:
        raise last_err

    # DRAM row r of each per-core output is token r of that core's slice
    # (the views write token p*1024+t at row p*1024+t), so a plain concat
    # along the token axis reassembles the full outputs.
    idx = np.concatenate([r["idx_out"] for r in res.results], axis=0)
    w = np.concatenate([r["w_out"] for r in res.results], axis=0)
    return idx.astype(np.int64), w.astype(np.float32)



# revision 4
# speedup vs baseline: 1.1854x; 1.1854x over previous
"""Trainium2 Bass kernel: MoE top-k router (top-8 of 64 experts + softmax).

Contract: kernel(logits, top_k) takes the FULL inputs (logits [1048576, 64]
f32, top_k == 8) and returns (topk_idx int64 [N, 8], topk_w f32 [N, 8]),
matching jax.lax.top_k + jax.nn.softmax semantics (stable descending order,
ties broken toward the smaller index).

Sharding: data-parallel over tokens across 8 NeuronCores (one SPMD program,
per-core slices fed via run_bass_kernel_spmd). Per core, tokens are laid out
partition-major — partition p owns tokens [p*1024, (p+1)*1024).

The top-8 selection runs as ONE hand-written custom DVE instruction per
[128, T, 64] tile (vs 3 stock match-unit instructions per 128 tokens): a
MAX8-style swap-flop MIN-cascade streams each token's 64 logits from SRC_0
(slice k's swap flop retains the (k+1)-th largest), then a FIND_INDEX8-style
IS_EQ match pass re-streams the same 64 SBUF words through the second read
port (SRC_1) latching each slice's match position, then 8 match indices and
8 values drain and the uOp chain loops to the next token. ~146 DVE cycles
per 128-token group instead of ~310. Indices drain as raw u32 bit patterns
into the f32 output tile; the host reinterprets them (match HW handles
duplicate values in jax tie order — verified on planted duplicates).

Softmax of the 8 selected values stays off the DVE: exp on ScalarE, the
denominator tree-sum and final scale on GPSIMD, reciprocal via the 1-cycle
DVE approx seed + Newton-Raphson refinement on GPSIMD (exact DVE reciprocal
on the final small tile to shorten the serial tail).
"""

import sys

if "/opt/trn_rl_repo" not in sys.path:
    sys.path.insert(0, "/opt/trn_rl_repo")

from dataclasses import dataclass

import numpy as np

N_TOKENS = 1048576
E = 64             # experts
K = 8              # top-k
NCORES = 8
P = 128            # SBUF partitions
TPC = N_TOKENS // NCORES   # tokens per core = 131072
TPP = TPC // P             # tokens per partition = 1024
T = 64                     # tokens per partition per full tile

_CACHE = {}


# --------------------------------------------------------------------------
# Custom fused top-8 DVE op (values + match indices in one instruction).
#
# uOp chain (intra-spec indices; tok_len = 64):
#   0: RAMP0 entry (elem 0 seeds stage-0 swap)   SRC_DONE->IDLE, CNT1->2
#   1: RAMP0 loop  (same config; loop target)    SRC_DONE->IDLE, CNT1->2
#   2..8: RAMP1..7 (MIN cascade j<k + seed k)    CNT1->next
#   9: STEADY_A    (8-stage MIN cascade)         CNT56->10
#  10: CLEAR       (clear_match bubble)          CNT1->11
#  11: STEADY_B    (IS_EQ(stream, swap), latch)  CNT64->12   [reads SRC_1]
#  12: SPACER      (pipeline flush bubble)       CNT1->13
#  13: IDX_DRAIN   (8x OutSel.MATCH_INDEX)       CNT8->14
#  14..21: VDRAIN s0..s7 (descending values)     CNT1->next; last->1 (loop)
# --------------------------------------------------------------------------

def _build_topk_uops(tok_len=64):
    from concourse.dve_uop import (
        AluInp, AluOp, InpSel, OutPath, OutSel, Trigger, UopConfig, ENABLE,
    )

    def ramp(k):
        u = UopConfig()
        u.enable_input(InpSel.SRC_0, 0)
        u.require_inp0 = ENABLE
        u.repeat_count = 1
        for j in range(k):
            u.datapath_config[j].enable_alu(
                AluOp.MIN, AluInp.CURR_SWAP_OUT, AluInp.PREV_ALU_OUT
            )
            u.datapath_config[j].swap_enable = ENABLE
        u.datapath_config[k].enable_alu(
            AluOp.BYPASS, AluInp.PREV_ALU_OUT, AluInp.PREV_ALU_OUT
        )
        u.datapath_config[k].swap_enable = ENABLE
        return u

    uops = []
    # Termination: src0's AP carries ONE extra element past the last token, so
    # ramp0 always has data to issue (it never stalls on requires_src0) and
    # the level-evaluated SRC_TENSOR_LT_8 fires on that issue cycle -> IDLE.
    # (Waiting stalled on SRC_TENSOR_DONE after the stream drained misses the
    # done event and leaves the uOp FSM stalled past instruction retirement,
    # wedging the engine for the next NEFF execution.)
    for _ in (0, 1):  # 0: entry, 1: loop re-entry (next_uop 0 means IDLE)
        u = ramp(0)
        u.trigger = (Trigger.SRC_TENSOR_LT_8, Trigger.COUNT, Trigger.NONE)
        u.next_uop = (0, 2, 0)
        uops.append(u)
    for k in range(1, 8):
        u = ramp(k)
        u.trigger = (Trigger.COUNT, Trigger.NONE, Trigger.NONE)
        u.next_uop = (k + 2, 0, 0)
        uops.append(u)

    u = UopConfig()  # 9: steady_A
    u.enable_input(InpSel.SRC_0, 0)
    u.require_inp0 = ENABLE
    u.repeat_count = tok_len - 8
    for j in range(8):
        u.datapath_config[j].enable_alu(
            AluOp.MIN, AluInp.PREV_ALU_OUT, AluInp.CURR_SWAP_OUT
        )
        u.datapath_config[j].swap_enable = ENABLE
    u.trigger = (Trigger.COUNT, Trigger.NONE, Trigger.NONE)
    u.next_uop = (10, 0, 0)
    uops.append(u)

    u = UopConfig()  # 10: clear_match bubble
    u.repeat_count = 1
    u.clear_match = ENABLE
    u.trigger = (Trigger.COUNT, Trigger.NONE, Trigger.NONE)
    u.next_uop = (11, 0, 0)
    uops.append(u)

    u = UopConfig()  # 11: steady_B — match pass over the second read port
    u.enable_input(InpSel.SRC_1, 1)  # lane 1 -> delay chain 0
    u.require_inp1 = ENABLE
    u.repeat_count = tok_len
    u.valid_match = ENABLE
    for j in range(8):
        u.datapath_config[j].enable_alu(
            AluOp.IS_EQ, AluInp.PREV_DELAY_0, AluInp.CURR_SWAP_OUT
        )
        u.datapath_config[j].pass_through_delay(0)
    u.trigger = (Trigger.COUNT, Trigger.NONE, Trigger.NONE)
    u.next_uop = (12, 0, 0)
    uops.append(u)

    u = UopConfig()  # 12: spacer
    u.repeat_count = 1
    u.trigger = (Trigger.COUNT, Trigger.NONE, Trigger.NONE)
    u.next_uop = (13, 0, 0)
    uops.append(u)

    u = UopConfig()  # 13: idx drain
    u.repeat_count = 8
    u.enable_output(OutSel.MATCH_INDEX, OutPath.WR0_LO)
    u.trigger = (Trigger.COUNT, Trigger.NONE, Trigger.NONE)
    u.next_uop = (14, 0, 0)
    uops.append(u)

    for k in range(8):  # 14..21: value drains, slice 0 (largest) first
        u = UopConfig()
        u.repeat_count = 1
        u.datapath_config[k].enable_alu(
            AluOp.BYPASS, AluInp.CURR_SWAP_OUT, AluInp.CURR_SWAP_OUT
        )
        for j in range(k + 1, 8):
            u.datapath_config[j].pass_through_alu()
        u.enable_output(OutSel.ALU_OUT, OutPath.WR0_LO)
        u.trigger = (Trigger.COUNT, Trigger.NONE, Trigger.NONE)
        u.next_uop = (15 + k if k < 7 else 1, 0, 0)
        uops.append(u)
    return uops


def _get_topk_op():
    if "op" in _CACHE:
        return _CACHE["op"]
    from concourse.dve_ops import (
        DveOp, OPS, CUSTOM_DVE_SPECS, _SUB_OPCODE_FOR_NAME, get_dve_sub_opcode,
    )
    from concourse.dve_spec import Spec, Src0, Src1
    from concourse.dve_uop import DveOpSpec

    uops = _build_topk_uops(E)
    # op name carries the uop-bytes hash: a uop edit changes the BIR and so
    # the NEFF cache key, preventing stale-table reuse.
    tag = DveOpSpec(name="probe", opcode=1, uops=uops, rd1_en=True).sha("v3")[:8]
    name = f"TOPK8_{tag}"

    @dataclass(frozen=True)
    class RawDveOp(DveOp):
        raw_uops: tuple = ()

        def compile(self, ver):
            assert ver == "v3", f"hand-written for TRN2/v3 only, got {ver}"
            return DveOpSpec(
                name=self.name,
                opcode=get_dve_sub_opcode(self.name),
                uops=list(self.raw_uops),
                rd1_en=True,
            )

    def _ref(in0, in1, s0, s1, imm2):
        # CoreSim-only; the HW path never calls this.
        p = in0.shape[0]
        x = in0.reshape(p, -1, E)
        t = x.shape[1]
        out = np.zeros((p, t * 16), dtype=np.float32)
        order = np.argsort(-x, axis=-1, kind="stable")[..., :8]
        vals = np.take_along_axis(x, order, axis=-1)
        out.reshape(p, t, 16)[:, :, 0:8] = order.astype(np.uint32).view(np.float32)
        out.reshape(p, t, 16)[:, :, 8:16] = vals
        return out

    op = RawDveOp(
        name=name,
        spec=Spec(body=Src0 + Src1, reference=_ref),
        subdim=False,
        uops_sha={},
        raw_uops=tuple(uops),
    )
    if name not in _SUB_OPCODE_FOR_NAME:
        row = max(_SUB_OPCODE_FOR_NAME.values()) + 1
        assert row < 0x20, f"row {row} overflows the 5-bit byte-36 field"
        OPS.append(op)
        CUSTOM_DVE_SPECS[op.name] = op.spec
        _SUB_OPCODE_FOR_NAME[op.name] = row
    _CACHE["op"] = op
    return op


def _build(tpp=TPP, t_tile=T):
    import concourse.bacc as bacc
    import concourse.mybir as mybir
    import concourse.tile as tile

    f32 = mybir.dt.float32
    op = _get_topk_op()

    n_tok = P * tpp
    # small first tiles (DVE starts after ~256KB of DMA instead of 2MB) and a
    # small last tile (short softmax/store tail after the final DVE instr)
    if tpp == 1024 and t_tile == 64:
        sizes = [8, 56] + [64] * 14 + [48, 16]
    else:
        sizes = [t_tile] * (tpp // t_tile)
    assert sum(sizes) == tpp
    offs = [sum(sizes[:j]) for j in range(len(sizes))]

    nc = bacc.Bacc("TRN2", target_bir_lowering=False, debug=False)
    logits = nc.dram_tensor("logits", [n_tok, E], f32, kind="ExternalInput")
    # idx leaves the device as raw u32 bit patterns in an f32 tensor; the
    # host reinterprets. w is the softmax over the 8 selected logits.
    idx_out = nc.dram_tensor("idx_out", [n_tok, K], f32, kind="ExternalOutput")
    w_out = nc.dram_tensor("w_out", [n_tok, K], f32, kind="ExternalOutput")

    lg_v = logits.ap().rearrange("(p t) e -> p t e", p=P, t=tpp)
    ix_v = idx_out.ap().rearrange("(p t) k -> p t k", p=P, t=tpp)
    w_v = w_out.ap().rearrange("(p t) k -> p t k", p=P, t=tpp)

    with tile.TileContext(nc) as tc:
        with tc.tile_pool(name="io", bufs=4) as pool:
            for o, tt in zip(offs, sizes):
                # one extra trailing element feeds the LT_8 termination issue
                x = pool.tile([P, tt * E + 1], f32, tag="x")
                x3 = x[:, 0:tt * E].rearrange("p (t e) -> p t e", t=tt, e=E)
                nc.sync.dma_start(x3, lg_v[:, o:o + tt, :])
                y = pool.tile([P, tt, 16], f32, tag="y")
                nc.vector._custom_dve(op, out=y[:], in0=x[:], in1=x3,
                                      s0=0.0, s1=0.0)
                vals = y[:, :, 8:16]
                ex = pool.tile([P, tt, K], f32, tag="ex")
                nc.scalar.activation(
                    ex[:], vals, mybir.ActivationFunctionType.Exp
                )
                # softmax denominator: pairwise tree-sum on GPSIMD
                t1 = pool.tile([P, tt, 4], f32, tag="t1")
                t2 = pool.tile([P, tt, 2], f32, tag="t2")
                s = pool.tile([P, tt, 1], f32, tag="s")
                nc.gpsimd.tensor_add(t1[:], ex[:, :, 0:4], ex[:, :, 4:8])
                nc.gpsimd.tensor_add(t2[:], t1[:, :, 0:2], t1[:, :, 2:4])
                nc.gpsimd.tensor_add(s[:], t2[:, :, 0:1], t2[:, :, 1:2])
                r = pool.tile([P, tt, 1], f32, tag="r")
                if o + tt == tpp:
                    # last tile: exact DVE reciprocal beats seed + 3 GPSIMD
                    # refinement ops on the serial tail
                    nc.vector.reciprocal(r[:], s[:])
                else:
                    rs = pool.tile([P, tt, 1], f32, tag="rs")
                    nc.vector.reciprocal_approx_fast(rs[:], s[:])
                    pq = pool.tile([P, tt, 1], f32, tag="pq")
                    nc.gpsimd.tensor_mul(pq[:], s[:], rs[:])
                    nc.gpsimd.tensor_scalar(pq[:], pq[:], -1.0, 2.0,
                                            op0=mybir.AluOpType.mult,
                                            op1=mybir.AluOpType.add)
                    nc.gpsimd.tensor_mul(r[:], rs[:], pq[:])
                w = pool.tile([P, tt, K], f32, tag="w")
                nc.gpsimd.tensor_mul(w[:], ex[:], r[:].broadcast_to([P, tt, K]))
                nc.sync.dma_start(ix_v[:, o:o + tt, :], y[:, :, 0:8])
                nc.sync.dma_start(w_v[:, o:o + tt, :], w[:])
    nc.compile()
    return nc


def _get_nc():
    if "nc" not in _CACHE:
        _CACHE["nc"] = _build()
    return _CACHE["nc"]


def kernel(logits, top_k):
    logits = np.asarray(logits, dtype=np.float32)
    k = int(np.asarray(top_k))
    assert k == K, f"kernel hardcodes top_k={K}, got {k}"
    assert logits.shape == (N_TOKENS, E), logits.shape

    from concourse.bass_utils import run_bass_kernel_spmd

    nc = _get_nc()
    chunks = logits.reshape(NCORES, TPC, E)
    in_maps = [{"logits": np.ascontiguousarray(chunks[c])} for c in range(NCORES)]
    # The tunneled devices occasionally fail a run with a transient
    # NRT_EXEC_UNIT_UNRECOVERABLE error; a straight retry recovers.
    last_err = None
    for _attempt in range(3):
        try:
            res = run_bass_kernel_spmd(nc, in_maps, list(range(NCORES)))
            break
        except Exception as e:  # noqa: BLE001 - retry transient device faults
            last_err = e
            import time as _time

            _time.sleep(5.0)
    else:
        raise last_err

    # Row r of each per-core output is token r of that core's slice, so a
    # plain concat along the token axis reassembles the full outputs.
    idx_f = np.concatenate([r["idx_out"] for r in res.results], axis=0)
    w = np.concatenate([r["w_out"] for r in res.results], axis=0)
    idx = np.ascontiguousarray(idx_f).view(np.uint32).astype(np.int64)
    return idx, w.astype(np.float32)
